# revision 1
# baseline (speedup 1.0000x reference)
"""Trainium2 Bass kernel for CustomRoPEAttention (B=2, S=2048, H=16, Dh=128).

Sharding: 8 cores = 2 batches x 4 head-groups (4 heads/core), tensor-parallel
over heads + data-parallel over batch. Each core computes QKV projection for
its heads (f32r matmuls), RoPE, causal softmax attention, and a partial
(transposed) output projection. Host sums the 4 partials per batch + bias.

Self-contained: hardcodes shapes from the problem spec.
"""
import math
from contextlib import ExitStack

import numpy as np

import concourse.mybir as mybir
import concourse.tile as tile
from concourse import bacc
from concourse.bass_utils import run_bass_kernel_spmd
from concourse.masks import make_identity

S = 2048            # sequence
D = 2048            # hidden
NH = 16             # total heads
DH = 128            # head dim
HG = 4              # heads per core
GQ = HG * DH        # 512: per-core q/k/v feature width
B = 2
NCORES = 8
ROPE_THETA = 10000.0
SCALE = 1.0 / math.sqrt(DH)
NEG = -1.0e9
SLAB = 256          # phase-1 sequence slab width
F32 = mybir.dt.float32
F32R = mybir.dt.float32r
MULT = mybir.AluOpType.mult
ADD = mybir.AluOpType.add


def build_nc(reps=1, phases=(1, 2, 3), knobs=None):
    kn = {"p1x": 4, "p1s": 4, "p2a": 4, "p2t": 2, "p2sp": 2, "p2tp": 3, "p2cp": 1, "p3ps": 4, "spw": 1024, "slab": SLAB, "p1ps": 3, "p1vps": 2, "splitw": 0, "vfirst": 0, "norope": 0, "nospill": 0, "atb": 4, "aev": 0, "wo_early": 1}
    if knobs:
        kn.update(knobs)
    nc = bacc.Bacc(None, target_bir_lowering=False)
    xt = nc.dram_tensor("xt", [16, 128, S], F32R, kind="ExternalInput")       # x^T tiles [kc,p,s]
    wqk = nc.dram_tensor("wqk", [16, 128, 2 * GQ], F32R, kind="ExternalInput")
    wv = nc.dram_tensor("wv", [16, 128, GQ], F32R, kind="ExternalInput")
    wo = nc.dram_tensor("wo", [4, 128, D], F32R, kind="ExternalInput")        # Wo rows tiles
    bqkt = nc.dram_tensor("bqkt", [128, 8], F32, kind="ExternalInput")        # q/k bias per (dh, mt)
    bqkt_sw = nc.dram_tensor("bqkt_sw", [128, 8], F32, kind="ExternalInput")   # same, halves swapped
    bv = nc.dram_tensor("bv", [1, GQ], F32, kind="ExternalInput")
    cost = nc.dram_tensor("cost", [128, S], F32, kind="ExternalInput")        # cos^T
    sinrt = nc.dram_tensor("sinrt", [128, S], F32, kind="ExternalInput")      # sin^T with rot sign
    maskd = nc.dram_tensor("maskd", [128, 128], F32, kind="ExternalInput")    # diag causal add-mask
    outt = nc.dram_tensor("outt", [16, 128, S], F32, kind="ExternalOutput")   # partial^T tiles
    qks = nc.dram_tensor("qks", [2 * HG, 128, S], F32R)                       # spill: q then k head tiles
    vsp = nc.dram_tensor("vsp", [16, 128, GQ], F32R)                          # spill: V natural tiles

    with tile.TileContext(nc) as tc, ExitStack() as top:
        g = top.enter_context(tc.tile_pool(name="glob", bufs=1))
        tcos = g.tile([128, S], F32)
        nc.sync.dma_start(out=tcos, in_=cost[:])
        tsin = g.tile([128, S], F32)
        nc.sync.dma_start(out=tsin, in_=sinrt[:])
        tmask = g.tile([128, 128], F32)
        nc.sync.dma_start(out=tmask, in_=maskd[:])
        ident_f = g.tile([128, 128], F32)
        make_identity(nc, ident_f[:])
        ident = g.tile([128, 128], F32R)
        nc.vector.tensor_copy(out=ident[:], in_=ident_f[:])
        tbqkt = g.tile([128, 8], F32)
        nc.sync.dma_start(out=tbqkt, in_=bqkt[:])
        tbqkt_sw = g.tile([128, 8], F32)
        nc.sync.dma_start(out=tbqkt_sw, in_=bqkt_sw[:])
        tbvb = g.tile([128, GQ], F32)
        nc.sync.dma_start(out=tbvb, in_=bv[:].to_broadcast((128, GQ)))
        for _rep in range(reps):
          if 1 in phases:
            # ---------------- Phase 1: QKV^T projection + RoPE + spill ----------------
            SLB = kn["slab"]
            with tc.tile_pool(name="p1w", bufs=1) as p1w, \
                 tc.tile_pool(name="p1x", bufs=kn["p1x"]) as p1x, \
                 tc.tile_pool(name="p1s", bufs=kn["p1s"]) as p1s, \
                 tc.tile_pool(name="p1ps", bufs=kn["p1ps"], space="PSUM") as p1ps, \
                 tc.tile_pool(name="p1vps", bufs=kn["p1vps"], space="PSUM") as p1vps:
                twqk = []
                twv = []
                for _kc in range(16):
                    wqkt = p1w.tile([128, 2 * GQ], F32R, tag=f"wqk{_kc}")
                    nc.sync.dma_start(out=wqkt, in_=wqk[_kc])
                    twqk.append(wqkt)
                    wvt = p1w.tile([128, GQ], F32R, tag=f"wv{_kc}")
                    nc.sync.dma_start(out=wvt, in_=wv[_kc])
                    twv.append(wvt)
                for ns in range(S // SLB):
                    sl = slice(ns * SLB, (ns + 1) * SLB)
                    xs = p1x.tile([128, 16, SLB], F32R, tag="xs")
                    nc.sync.dma_start(out=xs, in_=xt[:, :, sl].rearrange("kc p s -> p kc s"))
                    # Q^T and K^T head tiles (mt 0..3 = q heads, 4..7 = k heads)
                    for mt in range(2 * HG):
                        pqk = p1ps.tile([128, SLB], F32, tag="qkps")
                        for kc in range(16):
                            nc.tensor.matmul(pqk[:], twqk[kc][:, mt * 128:(mt + 1) * 128],
                                             xs[:, kc, :], start=(kc == 0), stop=(kc == 15))
                        if kn["norope"]:
                            qf = p1s.tile([128, SLB], F32R, tag="qf")
                            nc.scalar.copy(out=qf[:], in_=pqk[:])
                        else:
                            qraw = p1s.tile([128, SLB], F32, tag="qraw")
                            nc.scalar.copy(out=qraw[:], in_=pqk[:])
                            qsw = p1s.tile([128, SLB], F32, tag="qsw")
                            nc.sync.dma_start(out=qsw[0:64, :], in_=qraw[64:128, :])
                            nc.sync.dma_start(out=qsw[64:128, :], in_=qraw[0:64, :])
                            m1 = p1s.tile([128, SLB], F32R, tag="m1")
                            nc.vector.scalar_tensor_tensor(
                                out=m1[:], in0=pqk[:], scalar=tbqkt[:, mt:mt + 1],
                                in1=tcos[:, sl], op0=ADD, op1=MULT)
                            m2 = p1s.tile([128, SLB], F32R, tag="m2")
                            nc.vector.scalar_tensor_tensor(
                                out=m2[:], in0=qsw[:], scalar=tbqkt_sw[:, mt:mt + 1],
                                in1=tsin[:, sl], op0=ADD, op1=MULT)
                            qf = p1s.tile([128, SLB], F32R, tag="qf")
                            nc.vector.tensor_tensor(out=qf[:], in0=m1[:], in1=m2[:], op=ADD)
                        if not kn["nospill"]:
                            nc.sync.dma_start(out=qks[mt, :, sl], in_=qf[:])
                    # V natural tiles for this slab
                    for st in range(SLB // 128):
                        pv = p1vps.tile([128, GQ], F32, tag="vps")
                        s0 = st * 128
                        for kc in range(16):
                            nc.tensor.matmul(pv[:], xs[:, kc, s0:s0 + 128],
                                             twv[kc][:], start=(kc == 0), stop=(kc == 15))
                        vsb = p1s.tile([128, GQ], F32R, tag="vsb")
                        nc.vector.tensor_tensor(out=vsb[:], in0=pv[:], in1=tbvb[:], op=ADD)
                        if not kn["nospill"]:
                            nc.sync.dma_start(out=vsp[ns * (SLB // 128) + st], in_=vsb[:])

          # C^T [ (head,dh), S ] persists from phase 2 into phase 3
          ctstack = ExitStack()
          ctpool = ctstack.enter_context(tc.tile_pool(name="ctp", bufs=1))
          two_early = None
          if kn["wo_early"]:
              two_early = ctpool.tile([128, 4, D], F32R, tag="two_early")
              nc.sync.dma_start(out=two_early, in_=wo.rearrange("kc p f -> p kc f"))
          ct_sb = {}
          for _h in range(HG):
              for _q in range(4):
                  ctq = ctpool.tile([128, 512], F32R, tag=f"ct_{_h}_{_q}")
                  ct_sb[(_h, _q)] = ctq
          if 2 in phases:
            # ---------------- Phase 2: attention per head ----------------
            with tc.tile_pool(name="p2h", bufs=2) as p2h, \
                 tc.tile_pool(name="p2a", bufs=kn["p2a"]) as p2a, \
                 tc.tile_pool(name="p2t", bufs=kn["p2t"]) as p2t, \
                 tc.tile_pool(name="p2sp", bufs=kn["p2sp"], space="PSUM") as p2sp, \
                 tc.tile_pool(name="p2tp", bufs=kn["p2tp"], space="PSUM") as p2tp, \
                 tc.tile_pool(name="p2cp", bufs=kn["p2cp"], space="PSUM") as p2cp:
                for h in range(HG):
                    qh2, kh2, vh2 = [], [], []
                    for half in range(2):
                        qht = p2h.tile([128, 1024], F32R, tag=f"qh{half}")
                        nc.sync.dma_start(out=qht, in_=qks[h][:, half * 1024:(half + 1) * 1024])
                        qh2.append(qht)
                        kht = p2h.tile([128, 1024], F32R, tag=f"kh{half}")
                        nc.sync.dma_start(out=kht, in_=qks[HG + h][:, half * 1024:(half + 1) * 1024])
                        kh2.append(kht)
                        vht = p2h.tile([128, 8, 128], F32R, tag=f"vh{half}")
                        nc.sync.dma_start(
                            out=vht,
                            in_=vsp[half * 8:(half + 1) * 8, :,
                                    h * 128:(h + 1) * 128].rearrange("t p f -> p t f"))
                        vh2.append(vht)
                    for j in range(8):
                        at_sb = p2t.tile([128, 16, 256], F32R, tag="atsb")
                        for ii, i in enumerate((2 * j, 2 * j + 1)):
                            ski = (i + 1) * 128
                            spw = kn["spw"]
                            nchunk = (ski + spw - 1) // spw
                            ai = p2a.tile([128, S], F32R, tag="ai")
                            hs = p2a.tile([128, 4], F32, tag="hs")
                            for cc in range(nchunk):
                                off = cc * spw
                                w = min(spw, ski - off)
                                sp = p2sp.tile([128, spw], F32, tag="sp")
                                for s5 in range(0, w, 512):
                                    w5 = min(512, w - s5)
                                    ko = off + s5
                                    nc.tensor.matmul(
                                        sp[:, s5:s5 + w5],
                                        qh2[i // 8][:, (i % 8) * 128:(i % 8 + 1) * 128],
                                        kh2[ko // 1024][:, ko % 1024:ko % 1024 + w5],
                                        start=True, stop=True)
                                if off <= i * 128 < off + w:  # diagonal block lives here
                                    dd = i * 128 - off
                                    nc.vector.tensor_tensor(out=sp[:, dd:dd + 128],
                                                            in0=sp[:, dd:dd + 128],
                                                            in1=tmask[:], op=ADD)
                                nc.scalar.activation(out=ai[:, off:off + w], in_=sp[:, 0:w],
                                                     func=mybir.ActivationFunctionType.Exp,
                                                     scale=SCALE, accum_out=hs[:, cc:cc + 1])
                            for cc in range(1, nchunk):
                                nc.vector.tensor_tensor(out=hs[:, 0:1], in0=hs[:, 0:1],
                                                        in1=hs[:, cc:cc + 1], op=ADD)
                            rec = p2a.tile([128, 1], F32, tag="rec")
                            nc.vector.reciprocal(out=rec[:], in_=hs[:, 0:1])
                            nc.vector.tensor_tensor(out=ai[:, 0:ski], in0=ai[:, 0:ski],
                                                    in1=rec[:].broadcast_to((128, ski)), op=MULT)
                            ATB = kn["atb"]
                            for ks0 in range(0, i + 1, ATB):
                                nb = min(ATB, i + 1 - ks0)
                                atp = p2tp.tile([128, ATB, 128], F32R, tag="atp")
                                for t in range(nb):
                                    nc.tensor.transpose(atp[:, t, :],
                                                        ai[:, (ks0 + t) * 128:(ks0 + t + 1) * 128],
                                                        ident[:])
                                dst = at_sb[:, ks0:ks0 + nb, ii * 128:(ii + 1) * 128]
                                use_act = (kn["aev"] == 2 or
                                           (kn["aev"] == 0 and (ks0 // ATB + ii) % 2 == 0))
                                if use_act:
                                    nc.scalar.copy(out=dst, in_=atp[:, 0:nb, :])
                                else:
                                    nc.vector.tensor_copy(out=dst, in_=atp[:, 0:nb, :])
                        ct = p2cp.tile([128, 256], F32, tag="ct")
                        for ks in range(2 * j + 1):
                            nc.tensor.matmul(ct[:], vh2[ks // 8][:, ks % 8, :],
                                             at_sb[:, ks, :],
                                             start=(ks == 0), stop=False)
                        ksl = 2 * j + 1
                        nc.tensor.matmul(ct[:, 128:256], vh2[ksl // 8][:, ksl % 8, :],
                                         at_sb[:, ksl, 128:256], start=False, stop=True)
                        nc.scalar.copy(
                            out=ct_sb[(h, j // 2)][:, (j % 2) * 256:(j % 2 + 1) * 256],
                            in_=ct[:])

          if 3 in phases:
            # ---------------- Phase 3: output projection (transposed partial) ----------------
            with tc.tile_pool(name="p3w", bufs=1) as p3w, \
                 tc.tile_pool(name="p3s", bufs=4) as p3s, \
                 tc.tile_pool(name="p3ps", bufs=kn["p3ps"], space="PSUM") as p3ps:
                if two_early is not None:
                    two = two_early
                else:
                    two = p3w.tile([128, 4, D], F32R)
                    nc.sync.dma_start(out=two, in_=wo.rearrange("kc p f -> p kc f"))
                for ncc in range(4):
                    for mt in range(16):
                        op = p3ps.tile([128, 512], F32, tag="op")
                        for kh in range(4):
                            nc.tensor.matmul(op[:], two[:, kh, mt * 128:(mt + 1) * 128],
                                             ct_sb[(kh, ncc)][:],
                                             start=(kh == 0), stop=(kh == 3))
                        ob = p3s.tile([128, 512], F32, tag="ob")
                        if mt % 2 == 0:
                            nc.vector.tensor_copy(out=ob[:], in_=op[:])
                        else:
                            nc.scalar.copy(out=ob[:], in_=op[:])
                        nc.sync.dma_start(out=outt[mt, :, ncc * 512:(ncc + 1) * 512], in_=ob[:])
          ctstack.close()
    nc.finalize()
    return nc


_NC_CACHE = {}


def _get_nc(reps=1):
    if reps not in _NC_CACHE:
        _NC_CACHE[reps] = build_nc(reps)
    return _NC_CACHE[reps]


def _rope_tables(position_ids_b):
    pos = position_ids_b.astype(np.float32)
    inv_freq = (1.0 / (ROPE_THETA ** (np.arange(0, DH, 2, dtype=np.float32) / np.float32(DH))))
    ang = pos[:, None] * inv_freq[None, :]          # [S, 64]
    emb = np.concatenate([ang, ang], axis=-1)       # [S, 128]
    cosT = np.ascontiguousarray(np.cos(emb).T)      # [128, S]
    sinT = np.sin(emb).T
    sin_rot = np.concatenate([-sinT[0:64], sinT[64:128]], axis=0)
    return cosT.astype(np.float32), np.ascontiguousarray(sin_rot).astype(np.float32)


def _make_in_maps(inputs):
    hidden_states = np.asarray(inputs["hidden_states"], dtype=np.float32)
    position_ids = np.asarray(inputs["position_ids"])
    Wqkv = np.asarray(inputs["Wqkv"], dtype=np.float32)
    bqkv = np.asarray(inputs["bqkv"], dtype=np.float32)
    Wo = np.asarray(inputs["Wo"], dtype=np.float32)

    mask = np.triu(np.full((128, 128), NEG, dtype=np.float32), k=1)
    tabs = [_rope_tables(np.asarray(position_ids)[b]) for b in range(B)]
    xts = [np.ascontiguousarray(hidden_states[b].T).reshape(16, 128, S) for b in range(B)]

    in_maps = []
    for c in range(NCORES):
        b, hg = divmod(c, HG)
        qcols = slice(hg * GQ, (hg + 1) * GQ)
        kcols = slice(D + hg * GQ, D + (hg + 1) * GQ)
        vcols = slice(2 * D + hg * GQ, 2 * D + (hg + 1) * GQ)
        wqk_c = np.ascontiguousarray(
            np.concatenate([Wqkv[:, qcols], Wqkv[:, kcols]], axis=1)).reshape(16, 128, 2 * GQ)
        wv_c = np.ascontiguousarray(Wqkv[:, vcols]).reshape(16, 128, GQ)
        wo_c = np.ascontiguousarray(Wo[hg * GQ:(hg + 1) * GQ, :]).reshape(4, 128, D)
        bqk_c = np.concatenate([bqkv[qcols], bqkv[kcols]]).reshape(8, 128).T
        bqk_sw = np.concatenate([bqk_c[64:128], bqk_c[0:64]], axis=0)
        bv_c = bqkv[vcols].reshape(1, GQ)
        cosT, sin_rot = tabs[b]
        in_maps.append({
            "xt": xts[b], "wqk": wqk_c, "wv": wv_c, "wo": wo_c,
            "bqkt": np.ascontiguousarray(bqk_c), "bqkt_sw": np.ascontiguousarray(bqk_sw),
            "bv": np.ascontiguousarray(bv_c),
            "cost": cosT, "sinrt": sin_rot, "maskd": mask,
        })
    return in_maps


def kernel(hidden_states, position_ids, Wqkv, bqkv, Wo, bo, _reps=1):
    bo = np.asarray(bo, dtype=np.float32)
    in_maps = _make_in_maps({
        "hidden_states": hidden_states, "position_ids": position_ids,
        "Wqkv": Wqkv, "bqkv": bqkv, "Wo": Wo, "bo": bo,
    })
    nc = _get_nc(_reps)
    res = run_bass_kernel_spmd(nc, in_maps, core_ids=list(range(NCORES)))

    out = np.empty((B, S, D), dtype=np.float32)
    for b in range(B):
        acc = res.results[b * HG]["outt"].reshape(D, S).astype(np.float32).copy()
        for hg in range(1, HG):
            acc += res.results[b * HG + hg]["outt"].reshape(D, S)
        out[b] = acc.T + bo[None, :]
    return out



# revision 7
# speedup vs baseline: 1.4063x; 1.4063x over previous
"""Trainium2 Bass kernel for CustomRoPEAttention (B=2, S=2048, H=16, Dh=128).

Sharding: 8 cores = 2 batches x 4 head-groups (4 heads/core), tensor-parallel
over heads + data-parallel over batch. Each core computes QKV projection for
its heads, RoPE, causal softmax attention, and a partial (transposed) output
projection. Host sums the 4 partials per batch + bias.

V2 design notes:
- All matmul inputs are bf16 (psum accumulation stays f32), halving SBUF and
  DMA so q/k/v stay SBUF-resident between phases (no DRAM spill).
- Attention scores are computed transposed ([k, q] layout, k on partitions),
  so the exp'd scores feed the A@V matmul directly as lhsT with no PE
  transposes. A "ones" column in V (zero weight column + bias 1.0) produces
  softmax row-sums inside the A@V matmul; normalization happens after on DVE.
- RoPE rotate-half swaps are batched into 2 SBUF-to-SBUF DMAs per slab.
- Score chunks are emitted q-major so phase 2 can start before the last
  phase-1 slab retires; C^T transposes are deferred behind the next head's
  score matmuls to hide the normalize latency.

Self-contained: hardcodes shapes from the problem spec.
"""
import math
from contextlib import ExitStack

import numpy as np
import ml_dtypes

import concourse.mybir as mybir
import concourse.tile as tile
from concourse import bacc
from concourse.bass_utils import run_bass_kernel_spmd
from concourse.masks import make_identity

S = 2048            # sequence
D = 2048            # hidden
NH = 16             # total heads
DH = 128            # head dim
HG = 4              # heads per core
GQ = HG * DH        # 512: per-core q/k/v feature width
VW = 132            # per-head V storage width (128 + ones col + pad)
B = 2
NCORES = 8
NKB = S // 128      # 16 key blocks
ROPE_THETA = 10000.0
SCALE = 1.0 / math.sqrt(DH)
NEG = -1.0e9
F32 = mybir.dt.float32
BF16 = mybir.dt.bfloat16
MULT = mybir.AluOpType.mult
ADD = mybir.AluOpType.add
EXP = mybir.ActivationFunctionType.Exp
BF = ml_dtypes.bfloat16


def build_nc(knobs=None):
    kn = {"slab": 256, "p1x": 2, "p1ps": 4, "p1vps": 2, "p1m": 4,
          "p2sp": 3, "p2av": 3, "p2ct": 2, "p2e": 2, "p3ps": 4, "p3ob": 2,
          "schunk": 512}
    if knobs:
        kn.update(knobs)
    SLB = kn["slab"]
    NSLAB = S // SLB
    SCH = kn["schunk"]

    nc = bacc.Bacc(None, target_bir_lowering=False)
    xt = nc.dram_tensor("xt", [128, 16, S], BF16, kind="ExternalInput")    # x^T [p, kc, s]
    wqk = nc.dram_tensor("wqk", [128, 16, 2 * GQ], BF16, kind="ExternalInput")
    wv = nc.dram_tensor("wv", [128, 16, GQ], BF16, kind="ExternalInput")
    wo = nc.dram_tensor("wo", [128, 4, D], BF16, kind="ExternalInput")     # Wo rows [p, kh, f]
    bqkt = nc.dram_tensor("bqkt", [128, 8], F32, kind="ExternalInput")     # q/k bias per (dh, mt)
    bqkt_sw = nc.dram_tensor("bqkt_sw", [128, 8], F32, kind="ExternalInput")
    bv = nc.dram_tensor("bv", [1, GQ], F32, kind="ExternalInput")
    cost = nc.dram_tensor("cost", [128, S], F32, kind="ExternalInput")     # cos^T
    sinrt = nc.dram_tensor("sinrt", [128, S], F32, kind="ExternalInput")   # sin^T with rot sign
    maskd = nc.dram_tensor("maskd", [128, 128], F32, kind="ExternalInput") # [k,q] diag causal add-mask
    outt = nc.dram_tensor("outt", [16, 128, S], F32, kind="ExternalOutput")

    with tile.TileContext(nc) as tc, ExitStack() as top:
        g = top.enter_context(tc.tile_pool(name="glob", bufs=1))
        tcos = g.tile([128, S], F32, tag="tcos")
        tsin = g.tile([128, S], F32, tag="tsin")
        tmask = g.tile([128, 128], F32, tag="tmask")
        tbqkt = g.tile([128, 8], F32, tag="tbqkt")
        tbqkt_sw = g.tile([128, 8], F32, tag="tbqkt_sw")
        tbvb = g.tile([128, GQ], F32, tag="tbvb")
        two = g.tile([128, 4, D], BF16, tag="two")
        identf = g.tile([128, 128], F32, tag="identf")
        identb = g.tile([128, 128], BF16, tag="identb")

        # persistent q/k (transposed, [dh, s]) and v ([s, head, dh+ones])
        qk_sb = [g.tile([128, S], BF16, tag=f"qk{m}", name=f"qk{m}") for m in range(8)]
        vsb = [g.tile([128, HG, VW], BF16, tag=f"v{t}", name=f"v{t}") for t in range(NKB)]
        ct_sb = [g.tile([128, S], BF16, tag=f"ct{h}", name=f"ct{h}") for h in range(HG)]


        # ---------------- Phase 1: QKV^T projection + RoPE ----------------
        with tc.tile_pool(name="p1w", bufs=1) as p1w, \
             tc.tile_pool(name="p1x", bufs=kn["p1x"]) as p1x, \
             tc.tile_pool(name="p1r", bufs=2) as p1r, \
             tc.tile_pool(name="p1m", bufs=kn["p1m"]) as p1m, \
             tc.tile_pool(name="p1ps", bufs=kn["p1ps"], space="PSUM") as p1ps, \
             tc.tile_pool(name="p1vps", bufs=kn["p1vps"], space="PSUM") as p1vps:
            # weight loads chunked along output columns so the first q/k head
            # tiles can start as soon as chunk 0 + xs0 land
            twqk = p1w.tile([128, 16, 2 * GQ], BF16, tag="twqk")
            twv = p1w.tile([128, 16, GQ], BF16, tag="twv")
            # x slab 0 first: it plus wqk chunk 0 gate the very first matmul
            xs_pre = {0: p1x.tile([128, 16, SLB], BF16, tag="xs", name="xsp0")}
            nc.sync.dma_start(out=xs_pre[0], in_=xt[:, :, 0:SLB])
            nc.sync.dma_start(out=twqk[:, :, 0:256], in_=wqk[:, :, 0:256])
            nc.sync.dma_start(out=twqk[:, :, 256:512], in_=wqk[:, :, 256:512])
            nc.sync.dma_start(out=twv[:, :, 0:256], in_=wv[:, :, 0:256])
            nc.sync.dma_start(out=twqk[:, :, 512:768], in_=wqk[:, :, 512:768])
            nc.sync.dma_start(out=twqk[:, :, 768:1024], in_=wqk[:, :, 768:1024])
            nc.sync.dma_start(out=twv[:, :, 256:512], in_=wv[:, :, 256:512])
            nc.sync.dma_start(out=tcos, in_=cost[:])
            nc.sync.dma_start(out=tsin, in_=sinrt[:])
            nc.sync.dma_start(out=tbqkt, in_=bqkt[:])
            nc.sync.dma_start(out=tbqkt_sw, in_=bqkt_sw[:])
            nc.sync.dma_start(out=tbvb, in_=bv[:].to_broadcast((128, GQ)))
            nc.sync.dma_start(out=tmask, in_=maskd[:])
            nc.sync.dma_start(out=two, in_=wo[:])
            # prefetch slab 1 on the Pool queue before the memsets
            if NSLAB > 1:
                xs_pre[1] = p1x.tile([128, 16, SLB], BF16, tag="xs", name="xsp1")
                nc.gpsimd.dma_start(out=xs_pre[1], in_=xt[:, :, SLB:2 * SLB])
            # V ones columns are static: set once up front (Pool is otherwise idle)
            for t in range(NKB):
                nc.gpsimd.memset(vsb[t][:, :, 128:129], 1.0)
            make_identity(nc, identf[:])
            nc.vector.tensor_copy(out=identb[:], in_=identf[:])
            for ns in range(NSLAB):
                sl = slice(ns * SLB, (ns + 1) * SLB)
                if ns in xs_pre:
                    xs = xs_pre[ns]
                else:
                    xs = p1x.tile([128, 16, SLB], BF16, tag="xs")
                    nc.gpsimd.dma_start(out=xs, in_=xt[:, :, sl])
                qraw = p1r.tile([128, 8, SLB], F32, tag="qraw")
                qsw = p1r.tile([128, 8, SLB], F32, tag="qsw")
                # V natural tiles first (frees V psum early each slab)
                for st in range(SLB // 128):
                    pv = p1vps.tile([128, GQ], F32, tag="vps")
                    s0 = st * 128
                    for kc in range(16):
                        nc.tensor.matmul(pv[:], xs[:, kc, s0:s0 + 128],
                                         twv[:, kc, :], start=(kc == 0), stop=(kc == 15))
                    vt = vsb[ns * (SLB // 128) + st]
                    nc.vector.tensor_tensor(
                        out=vt[:, :, 0:128],
                        in0=pv[:].rearrange("p (h d) -> p h d", h=HG),
                        in1=tbvb[:].rearrange("p (h d) -> p h d", h=HG), op=ADD)
                # Q^T and K^T head tiles (mt 0..3 = q heads, 4..7 = k heads)
                for mt in range(2 * HG):
                    pqk = p1ps.tile([128, SLB], F32, tag="qkps")
                    for kc in range(16):
                        nc.tensor.matmul(pqk[:], twqk[:, kc, mt * 128:(mt + 1) * 128],
                                         xs[:, kc, :], start=(kc == 0), stop=(kc == 15))
                    nc.scalar.copy(out=qraw[:, mt, :], in_=pqk[:])
                # batched rotate-half swap (per-mt on the last slab so the
                # trailing RoPE chain drains with minimal latency)
                if ns == NSLAB - 1:
                    for mt in range(2 * HG):
                        nc.sync.dma_start(out=qsw[0:64, mt, :], in_=qraw[64:128, mt, :])
                        nc.sync.dma_start(out=qsw[64:128, mt, :], in_=qraw[0:64, mt, :])
                else:
                    nc.sync.dma_start(out=qsw[0:64, :, :], in_=qraw[64:128, :, :])
                    nc.sync.dma_start(out=qsw[64:128, :, :], in_=qraw[0:64, :, :])
                for mt in range(2 * HG):
                    m1 = p1m.tile([128, SLB], F32, tag="m1")
                    nc.vector.scalar_tensor_tensor(
                        out=m1[:], in0=qraw[:, mt, :], scalar=tbqkt[:, mt:mt + 1],
                        in1=tcos[:, sl], op0=ADD, op1=MULT)
                    m2 = p1m.tile([128, SLB], F32, tag="m2")
                    nc.vector.scalar_tensor_tensor(
                        out=m2[:], in0=qsw[:, mt, :], scalar=tbqkt_sw[:, mt:mt + 1],
                        in1=tsin[:, sl], op0=ADD, op1=MULT)
                    nc.vector.tensor_tensor(out=qk_sb[mt][:, sl], in0=m1[:], in1=m2[:],
                                            op=ADD)

        # ---------------- Phase 2: attention per head (scores transposed) ----------------
        with tc.tile_pool(name="p2e", bufs=kn["p2e"]) as p2e, \
             tc.tile_pool(name="p2c", bufs=4) as p2c, \
             tc.tile_pool(name="p2r", bufs=4) as p2r, \
             tc.tile_pool(name="p2sp", bufs=kn["p2sp"], space="PSUM") as p2sp, \
             tc.tile_pool(name="p2av", bufs=kn["p2av"], space="PSUM") as p2av, \
             tc.tile_pool(name="p2ct", bufs=kn["p2ct"], space="PSUM") as p2ct:

            def emit_scores(h):
                # q-major over 512-wide q chunks; within a chunk, k blocks
                qh = qk_sb[h]
                kh = qk_sb[HG + h]
                et = [p2e.tile([128, S - kb * 128], BF16, tag=f"e{kb}", name=f"e{kb}")
                      for kb in range(NKB)]
                for qc in range(0, S, SCH):
                    for kb in range(min(qc // 128 + SCH // 128, NKB)):
                        q0 = max(qc, kb * 128)
                        w = qc + SCH - q0
                        sp = p2sp.tile([128, SCH], F32, tag="sp")
                        nc.tensor.matmul(sp[:, 0:w], kh[:, kb * 128:(kb + 1) * 128],
                                         qh[:, q0:q0 + w], start=True, stop=True)
                        if q0 == kb * 128:  # diagonal block: causal mask add
                            nc.vector.tensor_tensor(out=sp[:, 0:128], in0=sp[:, 0:128],
                                                    in1=tmask[:], op=ADD)
                        nc.scalar.activation(out=et[kb][:, q0 - kb * 128:q0 - kb * 128 + w],
                                             in_=sp[:, 0:w], func=EXP, scale=SCALE)
                return et

            def emit_av(h, et):
                # A@V with ones column -> [q, dh | rowsum]; normalize on DVE
                cns = []
                for i in range(NKB):
                    av = p2av.tile([128, DH + 1], F32, tag="av")
                    for kb in range(i + 1):
                        nc.tensor.matmul(av[:], et[kb][:, (i - kb) * 128:(i - kb + 1) * 128],
                                         vsb[kb][:, h, 0:DH + 1],
                                         start=(kb == 0), stop=(kb == i))
                    rec = p2r.tile([128, 1], F32, tag="rec")
                    nc.vector.reciprocal(out=rec[:], in_=av[:, DH:DH + 1])
                    cn = p2c.tile([128, 128], BF16, tag="cn")
                    nc.vector.tensor_scalar_mul(cn[:], av[:, 0:DH], rec[:])
                    cns.append(cn)
                return cns

            def emit_ct(h, cns):
                # transpose normalized C into ct_sb[h]
                for i in range(NKB):
                    ctp = p2ct.tile([128, 128], BF16, tag="ctp")
                    nc.tensor.transpose(ctp[:], cns[i][:], identb[:])
                    nc.vector.tensor_copy(out=ct_sb[h][:, i * 128:(i + 1) * 128],
                                          in_=ctp[:])

            et = emit_scores(0)
            for h in range(HG):
                cns = emit_av(h, et)
                et = emit_scores(h + 1) if h + 1 < HG else None
                emit_ct(h, cns)

        # ---------------- Phase 3: output projection (transposed partial) ----------------
        with tc.tile_pool(name="p3ob", bufs=kn["p3ob"]) as p3ob, \
             tc.tile_pool(name="p3ps", bufs=kn["p3ps"], space="PSUM") as p3ps:
            for mt in range(16):
                ob = p3ob.tile([128, S], F32, tag="ob")
                for ncc in range(4):
                    op = p3ps.tile([128, 512], F32, tag="op")
                    for kh in range(4):
                        nc.tensor.matmul(op[:], two[:, kh, mt * 128:(mt + 1) * 128],
                                         ct_sb[kh][:, ncc * 512:(ncc + 1) * 512],
                                         start=(kh == 0), stop=(kh == 3))
                    if ncc % 2 == 0:
                        nc.vector.tensor_copy(out=ob[:, ncc * 512:(ncc + 1) * 512], in_=op[:])
                    else:
                        nc.scalar.copy(out=ob[:, ncc * 512:(ncc + 1) * 512], in_=op[:])
                    if mt == 15:
                        nc.sync.dma_start(out=outt[mt, :, ncc * 512:(ncc + 1) * 512],
                                          in_=ob[:, ncc * 512:(ncc + 1) * 512])
                if mt < 15:
                    nc.sync.dma_start(out=outt[mt], in_=ob[:])
    nc.finalize()
    return nc


_NC_CACHE = {}


def _get_nc(key=0, knobs=None):
    if key not in _NC_CACHE:
        _NC_CACHE[key] = build_nc(knobs)
    return _NC_CACHE[key]


def _rope_tables(position_ids_b):
    pos = position_ids_b.astype(np.float32)
    inv_freq = (1.0 / (ROPE_THETA ** (np.arange(0, DH, 2, dtype=np.float32) / np.float32(DH))))
    ang = pos[:, None] * inv_freq[None, :]          # [S, 64]
    emb = np.concatenate([ang, ang], axis=-1)       # [S, 128]
    cosT = np.ascontiguousarray(np.cos(emb).T)      # [128, S]
    sinT = np.sin(emb).T
    sin_rot = np.concatenate([-sinT[0:64], sinT[64:128]], axis=0)
    return cosT.astype(np.float32), np.ascontiguousarray(sin_rot).astype(np.float32)


def _make_in_maps(inputs):
    hidden_states = np.asarray(inputs["hidden_states"], dtype=np.float32)
    position_ids = np.asarray(inputs["position_ids"])
    Wqkv = np.asarray(inputs["Wqkv"], dtype=np.float32)
    bqkv = np.asarray(inputs["bqkv"], dtype=np.float32)
    Wo = np.asarray(inputs["Wo"], dtype=np.float32)

    # transposed diag mask: [k, q], NEG where q < k
    mask = np.tril(np.full((128, 128), NEG, dtype=np.float32), k=-1)
    tabs = [_rope_tables(np.asarray(position_ids)[b]) for b in range(B)]
    # x^T as [p, kc, s] bf16
    xts = []
    for b in range(B):
        xtb = np.ascontiguousarray(hidden_states[b].T).reshape(16, 128, S)
        xts.append(np.ascontiguousarray(xtb.transpose(1, 0, 2)).astype(BF))

    in_maps = []
    for c in range(NCORES):
        b, hg = divmod(c, HG)
        qcols = slice(hg * GQ, (hg + 1) * GQ)
        kcols = slice(D + hg * GQ, D + (hg + 1) * GQ)
        vcols = slice(2 * D + hg * GQ, 2 * D + (hg + 1) * GQ)
        wqk_c = np.concatenate([Wqkv[:, qcols], Wqkv[:, kcols]], axis=1) \
            .reshape(16, 128, 2 * GQ).transpose(1, 0, 2)
        wv_c = Wqkv[:, vcols].reshape(16, 128, GQ).transpose(1, 0, 2)
        wo_c = Wo[hg * GQ:(hg + 1) * GQ, :].reshape(4, 128, D).transpose(1, 0, 2)
        bqk_c = np.concatenate([bqkv[qcols], bqkv[kcols]]).reshape(8, 128).T
        bqk_sw = np.concatenate([bqk_c[64:128], bqk_c[0:64]], axis=0)
        bv_c = bqkv[vcols].reshape(1, GQ)
        cosT, sin_rot = tabs[b]
        in_maps.append({
            "xt": xts[b],
            "wqk": np.ascontiguousarray(wqk_c).astype(BF),
            "wv": np.ascontiguousarray(wv_c).astype(BF),
            "wo": np.ascontiguousarray(wo_c).astype(BF),
            "bqkt": np.ascontiguousarray(bqk_c), "bqkt_sw": np.ascontiguousarray(bqk_sw),
            "bv": np.ascontiguousarray(bv_c),
            "cost": cosT, "sinrt": sin_rot, "maskd": mask,
        })
    return in_maps


def kernel(hidden_states, position_ids, Wqkv, bqkv, Wo, bo):
    bo = np.asarray(bo, dtype=np.float32)
    in_maps = _make_in_maps({
        "hidden_states": hidden_states, "position_ids": position_ids,
        "Wqkv": Wqkv, "bqkv": bqkv, "Wo": Wo, "bo": bo,
    })
    nc = _get_nc()
    res = run_bass_kernel_spmd(nc, in_maps, core_ids=list(range(NCORES)))

    out = np.empty((B, S, D), dtype=np.float32)
    for b in range(B):
        acc = res.results[b * HG]["outt"].reshape(D, S).astype(np.float32).copy()
        for hg in range(1, HG):
            acc += res.results[b * HG + hg]["outt"].reshape(D, S)
        out[b] = acc.T + bo[None, :]
    return out


# revision 13
# speedup vs baseline: 1.4624x; 1.0399x over previous
"""Trainium2 Bass kernel for CustomRoPEAttention (B=2, S=2048, H=16, Dh=128).

Sharding: 8 cores = 2 batches x 4 head-groups (4 heads/core), tensor-parallel
over heads + data-parallel over batch. Each core computes QKV projection for
its heads, RoPE, causal softmax attention, and a partial (transposed) output
projection. Host sums the 4 partials per batch + bias.

V2 design notes:
- All matmul inputs are bf16 (psum accumulation stays f32), halving SBUF and
  DMA so q/k/v stay SBUF-resident between phases (no DRAM spill).
- Attention scores are computed transposed ([k, q] layout, k on partitions),
  so the exp'd scores feed the A@V matmul directly as lhsT with no PE
  transposes. A "ones" column in V (zero weight column + bias 1.0) produces
  softmax row-sums inside the A@V matmul; normalization happens after on DVE.
- RoPE rotate-half swaps are batched into 2 SBUF-to-SBUF DMAs per slab.
- Score chunks are emitted q-major so phase 2 can start before the last
  phase-1 slab retires; C^T transposes are deferred behind the next head's
  score matmuls to hide the normalize latency.

Self-contained: hardcodes shapes from the problem spec.
"""
import math
from contextlib import ExitStack

import numpy as np
import ml_dtypes

import concourse.mybir as mybir
import concourse.tile as tile
from concourse import bacc
from concourse.bass_utils import run_bass_kernel_spmd
from concourse.masks import make_identity

S = 2048            # sequence
D = 2048            # hidden
NH = 16             # total heads
DH = 128            # head dim
HG = 4              # heads per core
GQ = HG * DH        # 512: per-core q/k/v feature width
VW = 132            # per-head V storage width (128 + ones col + pad)
B = 2
NCORES = 8
NKB = S // 128      # 16 key blocks
ROPE_THETA = 10000.0
SCALE = 1.0 / math.sqrt(DH)
NEG = -1.0e9
F32 = mybir.dt.float32
BF16 = mybir.dt.bfloat16
MULT = mybir.AluOpType.mult
ADD = mybir.AluOpType.add
EXP = mybir.ActivationFunctionType.Exp
BF = ml_dtypes.bfloat16


def build_nc(knobs=None):
    kn = {"slab": 256, "p1x": 2, "p1ps": 5, "p1vps": 2, "p1m": 6,
          "p2sp": 4, "p2av": 3, "p2ct": 1, "p2e": 2, "p3ps": 4, "p3ob": 2,
          "schunk": 512}
    if knobs:
        kn.update(knobs)
    SLB = kn["slab"]
    NSLAB = S // SLB
    SCH = kn["schunk"]

    nc = bacc.Bacc(None, target_bir_lowering=False)
    xt = nc.dram_tensor("xt", [128, 16, S], BF16, kind="ExternalInput")    # x^T [p, kc, s]
    wqk = nc.dram_tensor("wqk", [128, 16, 2 * GQ], BF16, kind="ExternalInput")
    wv = nc.dram_tensor("wv", [128, 16, GQ], BF16, kind="ExternalInput")
    wo = nc.dram_tensor("wo", [128, 4, D], BF16, kind="ExternalInput")     # Wo rows [p, kh, f]
    bqkt = nc.dram_tensor("bqkt", [128, 8], F32, kind="ExternalInput")     # q/k bias per (dh, mt)
    bqkt_sw = nc.dram_tensor("bqkt_sw", [128, 8], F32, kind="ExternalInput")
    bv = nc.dram_tensor("bv", [1, GQ], F32, kind="ExternalInput")
    cost = nc.dram_tensor("cost", [128, S], BF16, kind="ExternalInput")     # cos^T
    sinrt = nc.dram_tensor("sinrt", [128, S], BF16, kind="ExternalInput")   # sin^T with rot sign
    outt = nc.dram_tensor("outt", [16, 128, S], BF16, kind="ExternalOutput")

    with tile.TileContext(nc) as tc, ExitStack() as top:
        g = top.enter_context(tc.tile_pool(name="glob", bufs=1))
        tcos = g.tile([128, S], BF16, tag="tcos")
        tsin = g.tile([128, S], BF16, tag="tsin")
        tbqkt = g.tile([128, 8], F32, tag="tbqkt")
        tbqkt_sw = g.tile([128, 8], F32, tag="tbqkt_sw")
        tbvb = g.tile([128, GQ], F32, tag="tbvb")
        identf = g.tile([128, 128], F32, tag="identf")
        identb = g.tile([128, 128], BF16, tag="identb")

        # persistent q/k (transposed, [dh, s]) and v ([s, head, dh+ones])
        qk_sb = [g.tile([128, S], BF16, tag=f"qk{m}", name=f"qk{m}") for m in range(8)]
        e_sb = [g.tile([128, S - kb * 128], BF16, tag=f"e{kb}", name=f"e{kb}")
                for kb in range(NKB)]
        vsb = [g.tile([128, HG, VW], BF16, tag=f"v{t}", name=f"v{t}") for t in range(NKB)]
        ct_sb = [g.tile([128, S], BF16, tag=f"ct{h}", name=f"ct{h}") for h in range(HG)]


        # ---------------- Phase 1: QKV^T projection + RoPE ----------------
        with tc.tile_pool(name="p1w", bufs=1) as p1w, \
             tc.tile_pool(name="p1x", bufs=kn["p1x"]) as p1x, \
             tc.tile_pool(name="p1r", bufs=2) as p1r, \
             tc.tile_pool(name="p1m", bufs=kn["p1m"]) as p1m, \
             tc.tile_pool(name="p1ps", bufs=kn["p1ps"], space="PSUM") as p1ps, \
             tc.tile_pool(name="p1vps", bufs=kn["p1vps"], space="PSUM") as p1vps:
            # weight loads chunked along output columns so the first q/k head
            # tiles can start as soon as chunk 0 + xs0 land
            twqk = p1w.tile([128, 16, 2 * GQ], BF16, tag="twqk")
            twv = p1w.tile([128, 16, GQ], BF16, tag="twv")
            # x slab 0 + first weight chunk land first, split by kc-half so
            # the very first matmul group can start ~4us earlier
            xs_pre = {0: p1x.tile([128, 16, SLB], BF16, tag="xs", name="xsp0")}
            nc.sync.dma_start(out=xs_pre[0][:, 0:8, :], in_=xt[:, 0:8, 0:SLB])
            nc.sync.dma_start(out=twqk[:, 0:8, 0:256], in_=wqk[:, 0:8, 0:256])
            nc.sync.dma_start(out=xs_pre[0][:, 8:16, :], in_=xt[:, 8:16, 0:SLB])
            nc.sync.dma_start(out=twqk[:, 8:16, 0:256], in_=wqk[:, 8:16, 0:256])
            nc.sync.dma_start(out=twqk[:, :, 256:512], in_=wqk[:, :, 256:512])
            nc.sync.dma_start(out=twv[:, :, 0:256], in_=wv[:, :, 0:256])
            nc.sync.dma_start(out=twqk[:, :, 512:768], in_=wqk[:, :, 512:768])
            nc.sync.dma_start(out=twqk[:, :, 768:1024], in_=wqk[:, :, 768:1024])
            nc.sync.dma_start(out=twv[:, :, 256:512], in_=wv[:, :, 256:512])
            nc.sync.dma_start(out=tcos, in_=cost[:])
            nc.sync.dma_start(out=tsin, in_=sinrt[:])
            nc.sync.dma_start(out=tbqkt, in_=bqkt[:])
            nc.sync.dma_start(out=tbqkt_sw, in_=bqkt_sw[:])
            nc.sync.dma_start(out=tbvb, in_=bv[:].to_broadcast((128, GQ)))
            # prefetch slab 1 on the Pool queue before the memsets
            if NSLAB > 1:
                xs_pre[1] = p1x.tile([128, 16, SLB], BF16, tag="xs", name="xsp1")
                nc.gpsimd.dma_start(out=xs_pre[1], in_=xt[:, :, SLB:2 * SLB])
            # V ones columns are static: set once up front (Pool is otherwise idle)
            for t in range(NKB):
                nc.gpsimd.memset(vsb[t][:, :, 128:129], 1.0)
            make_identity(nc, identf[:])
            nc.vector.tensor_copy(out=identb[:], in_=identf[:])
            for ns in range(NSLAB):
                sl = slice(ns * SLB, (ns + 1) * SLB)
                if ns in xs_pre:
                    xs = xs_pre[ns]
                else:
                    xs = p1x.tile([128, 16, SLB], BF16, tag="xs")
                    nc.gpsimd.dma_start(out=xs, in_=xt[:, :, sl])
                qraw = p1r.tile([128, 8, SLB], BF16, tag="qraw")
                qsw = p1r.tile([128, 8, SLB], BF16, tag="qsw")
                # V natural tiles first (frees V psum early each slab);
                # slab 0 runs QK first since wv lands after wqk
                for st in ([] if ns == 0 else range(SLB // 128)):
                    pv = p1vps.tile([128, GQ], F32, tag="vps")
                    s0 = st * 128
                    for kc in range(16):
                        nc.tensor.matmul(pv[:], xs[:, kc, s0:s0 + 128],
                                         twv[:, kc, :], start=(kc == 0), stop=(kc == 15))
                    vt = vsb[ns * (SLB // 128) + st]
                    nc.vector.tensor_tensor(
                        out=vt[:, :, 0:128],
                        in0=pv[:].rearrange("p (h d) -> p h d", h=HG),
                        in1=tbvb[:].rearrange("p (h d) -> p h d", h=HG), op=ADD)
                # Q^T and K^T head tiles (mt 0..3 = q heads, 4..7 = k heads)
                for mt in range(2 * HG):
                    pqk = p1ps.tile([128, SLB], F32, tag="qkps")
                    for kc in range(16):
                        nc.tensor.matmul(pqk[:], twqk[:, kc, mt * 128:(mt + 1) * 128],
                                         xs[:, kc, :], start=(kc == 0), stop=(kc == 15))
                    nc.scalar.copy(out=qraw[:, mt, :], in_=pqk[:])
                # batched rotate-half swap (per-mt on the last slab so the
                # trailing RoPE chain drains with minimal latency)
                if ns == NSLAB - 1:
                    for mt in range(2 * HG):
                        nc.sync.dma_start(out=qsw[0:64, mt, :], in_=qraw[64:128, mt, :])
                        nc.sync.dma_start(out=qsw[64:128, mt, :], in_=qraw[0:64, mt, :])
                else:
                    nc.sync.dma_start(out=qsw[0:64, :, :], in_=qraw[64:128, :, :])
                    nc.sync.dma_start(out=qsw[64:128, :, :], in_=qraw[0:64, :, :])
                for mt in range(2 * HG):
                    m1 = p1m.tile([128, SLB], BF16, tag="m1")
                    nc.vector.scalar_tensor_tensor(
                        out=m1[:], in0=qraw[:, mt, :], scalar=tbqkt[:, mt:mt + 1],
                        in1=tcos[:, sl], op0=ADD, op1=MULT)
                    m2 = p1m.tile([128, SLB], BF16, tag="m2")
                    nc.vector.scalar_tensor_tensor(
                        out=m2[:], in0=qsw[:, mt, :], scalar=tbqkt_sw[:, mt:mt + 1],
                        in1=tsin[:, sl], op0=ADD, op1=MULT)
                    nc.vector.tensor_tensor(out=qk_sb[mt][:, sl], in0=m1[:], in1=m2[:],
                                            op=ADD)
                if ns == 0:
                    for st in range(SLB // 128):
                        pv = p1vps.tile([128, GQ], F32, tag="vps")
                        s0 = st * 128
                        for kc in range(16):
                            nc.tensor.matmul(pv[:], xs[:, kc, s0:s0 + 128],
                                             twv[:, kc, :], start=(kc == 0), stop=(kc == 15))
                        vt = vsb[st]
                        nc.vector.tensor_tensor(
                            out=vt[:, :, 0:128],
                            in0=pv[:].rearrange("p (h d) -> p h d", h=HG),
                            in1=tbvb[:].rearrange("p (h d) -> p h d", h=HG), op=ADD)

        # ---------------- Phase 2: attention per head (scores transposed) ----------------
        late = top.enter_context(tc.tile_pool(name="late", bufs=1))
        two = late.tile([128, 4, D], BF16, tag="two")
        nc.sync.dma_start(out=two, in_=wo[:])
        with tc.tile_pool(name="p2c", bufs=4) as p2c, \
             tc.tile_pool(name="p2r", bufs=4) as p2r, \
             tc.tile_pool(name="p2sp", bufs=kn["p2sp"], space="PSUM") as p2sp, \
             tc.tile_pool(name="p2av", bufs=kn["p2av"], space="PSUM") as p2av, \
             tc.tile_pool(name="p2ct", bufs=kn["p2ct"], space="PSUM") as p2ct:

            def emit_scores(h):
                # q-major over 512-wide q chunks; within a chunk, k blocks
                qh = qk_sb[h]
                kh = qk_sb[HG + h]
                et = e_sb
                for qc in range(0, S, SCH):
                    for kb in range(min(qc // 128 + SCH // 128, NKB)):
                        q0 = max(qc, kb * 128)
                        w = qc + SCH - q0
                        sp = p2sp.tile([128, SCH], F32, tag="sp")
                        nc.tensor.matmul(sp[:, 0:w], kh[:, kb * 128:(kb + 1) * 128],
                                         qh[:, q0:q0 + w], start=True, stop=True)
                        nc.scalar.activation(out=et[kb][:, q0 - kb * 128:q0 - kb * 128 + w],
                                             in_=sp[:, 0:w], func=EXP, scale=SCALE)
                        if q0 == kb * 128:
                            # zero the causal-invalid triangle (q < k) on Pool:
                            # keeps exp off the DVE critical path entirely
                            nc.gpsimd.affine_select(
                                out=et[kb][:, 0:128], in_=et[kb][:, 0:128],
                                pattern=[[1, 128]], compare_op=mybir.AluOpType.is_ge,
                                fill=0.0, base=0, channel_multiplier=-1)
                return et

            def emit_av(h, et):
                # A@V with ones column -> [q, dh | rowsum]; normalize on DVE
                cns = []
                for i in range(NKB):
                    av = p2av.tile([128, DH + 1], F32, tag="av")
                    for kb in range(i + 1):
                        nc.tensor.matmul(av[:], et[kb][:, (i - kb) * 128:(i - kb + 1) * 128],
                                         vsb[kb][:, h, 0:DH + 1],
                                         start=(kb == 0), stop=(kb == i))
                    rec = p2r.tile([128, 1], F32, tag="rec")
                    nc.vector.reciprocal(out=rec[:], in_=av[:, DH:DH + 1])
                    cn = p2c.tile([128, 128], BF16, tag="cn")
                    nc.vector.tensor_scalar_mul(cn[:], av[:, 0:DH], rec[:])
                    cns.append(cn)
                return cns

            def emit_ct(h, cns):
                # transpose normalized C into ct_sb[h]
                for i in range(NKB):
                    ctp = p2ct.tile([128, 128], BF16, tag="ctp")
                    nc.tensor.transpose(ctp[:], cns[i][:], identb[:])
                    nc.vector.tensor_copy(out=ct_sb[h][:, i * 128:(i + 1) * 128],
                                          in_=ctp[:])

            et = emit_scores(0)
            for h in range(HG):
                cns = emit_av(h, et)
                et = emit_scores(h + 1) if h + 1 < HG else None
                emit_ct(h, cns)

        # ---------------- Phase 3: output projection (transposed partial) ----------------
        with tc.tile_pool(name="p3ob", bufs=kn["p3ob"]) as p3ob, \
             tc.tile_pool(name="p3ps", bufs=kn["p3ps"], space="PSUM") as p3ps:
            for mt in range(16):
                ob = p3ob.tile([128, S], BF16, tag="ob")
                for ncc in range(4):
                    op = p3ps.tile([128, 512], F32, tag="op")
                    for kh in range(4):
                        nc.tensor.matmul(op[:], two[:, kh, mt * 128:(mt + 1) * 128],
                                         ct_sb[kh][:, ncc * 512:(ncc + 1) * 512],
                                         start=(kh == 0), stop=(kh == 3))
                    if ncc % 2 == 0:
                        nc.vector.tensor_copy(out=ob[:, ncc * 512:(ncc + 1) * 512], in_=op[:])
                    else:
                        nc.scalar.copy(out=ob[:, ncc * 512:(ncc + 1) * 512], in_=op[:])
                    if mt == 15:
                        nc.sync.dma_start(out=outt[mt, :, ncc * 512:(ncc + 1) * 512],
                                          in_=ob[:, ncc * 512:(ncc + 1) * 512])
                if mt < 15:
                    nc.sync.dma_start(out=outt[mt], in_=ob[:])
    nc.finalize()
    return nc


_NC_CACHE = {}


def _get_nc(key=0, knobs=None):
    if key not in _NC_CACHE:
        _NC_CACHE[key] = build_nc(knobs)
    return _NC_CACHE[key]


def _rope_tables(position_ids_b):
    pos = position_ids_b.astype(np.float32)
    inv_freq = (1.0 / (ROPE_THETA ** (np.arange(0, DH, 2, dtype=np.float32) / np.float32(DH))))
    ang = pos[:, None] * inv_freq[None, :]          # [S, 64]
    emb = np.concatenate([ang, ang], axis=-1)       # [S, 128]
    cosT = np.ascontiguousarray(np.cos(emb).T)      # [128, S]
    sinT = np.sin(emb).T
    sin_rot = np.concatenate([-sinT[0:64], sinT[64:128]], axis=0)
    return cosT.astype(BF), np.ascontiguousarray(sin_rot).astype(BF)


def _make_in_maps(inputs):
    hidden_states = np.asarray(inputs["hidden_states"], dtype=np.float32)
    position_ids = np.asarray(inputs["position_ids"])
    Wqkv = np.asarray(inputs["Wqkv"], dtype=np.float32)
    bqkv = np.asarray(inputs["bqkv"], dtype=np.float32)
    Wo = np.asarray(inputs["Wo"], dtype=np.float32)

    tabs = [_rope_tables(np.asarray(position_ids)[b]) for b in range(B)]
    # x^T as [p, kc, s] bf16
    xts = []
    for b in range(B):
        xtb = np.ascontiguousarray(hidden_states[b].T).reshape(16, 128, S)
        xts.append(np.ascontiguousarray(xtb.transpose(1, 0, 2)).astype(BF))

    in_maps = []
    for c in range(NCORES):
        b, hg = divmod(c, HG)
        qcols = slice(hg * GQ, (hg + 1) * GQ)
        kcols = slice(D + hg * GQ, D + (hg + 1) * GQ)
        vcols = slice(2 * D + hg * GQ, 2 * D + (hg + 1) * GQ)
        wqk_c = np.concatenate([Wqkv[:, qcols], Wqkv[:, kcols]], axis=1) \
            .reshape(16, 128, 2 * GQ).transpose(1, 0, 2)
        wv_c = Wqkv[:, vcols].reshape(16, 128, GQ).transpose(1, 0, 2)
        wo_c = Wo[hg * GQ:(hg + 1) * GQ, :].reshape(4, 128, D).transpose(1, 0, 2)
        bqk_c = np.concatenate([bqkv[qcols], bqkv[kcols]]).reshape(8, 128).T
        bqk_sw = np.concatenate([bqk_c[64:128], bqk_c[0:64]], axis=0)
        bv_c = bqkv[vcols].reshape(1, GQ)
        cosT, sin_rot = tabs[b]
        in_maps.append({
            "xt": xts[b],
            "wqk": np.ascontiguousarray(wqk_c).astype(BF),
            "wv": np.ascontiguousarray(wv_c).astype(BF),
            "wo": np.ascontiguousarray(wo_c).astype(BF),
            "bqkt": np.ascontiguousarray(bqk_c), "bqkt_sw": np.ascontiguousarray(bqk_sw),
            "bv": np.ascontiguousarray(bv_c),
            "cost": cosT, "sinrt": sin_rot,
        })
    return in_maps


def kernel(hidden_states, position_ids, Wqkv, bqkv, Wo, bo):
    bo = np.asarray(bo, dtype=np.float32)
    in_maps = _make_in_maps({
        "hidden_states": hidden_states, "position_ids": position_ids,
        "Wqkv": Wqkv, "bqkv": bqkv, "Wo": Wo, "bo": bo,
    })
    nc = _get_nc()
    res = run_bass_kernel_spmd(nc, in_maps, core_ids=list(range(NCORES)))

    out = np.empty((B, S, D), dtype=np.float32)
    for b in range(B):
        acc = res.results[b * HG]["outt"].reshape(D, S).astype(np.float32).copy()
        for hg in range(1, HG):
            acc += res.results[b * HG + hg]["outt"].reshape(D, S)
        out[b] = acc.T + bo[None, :]
    return out
